# revision 6
# baseline (speedup 1.0000x reference)
import sys

sys.path.insert(0, "/opt/trn_rl_repo")

import numpy as np
import ml_dtypes

from concourse import bacc, mybir, tile
from concourse import bass_utils

# Problem constants (hardcoded)
NUM_USERS, NUM_ITEMS, NUM_REL = 100000, 50000, 8
EMB, LSTM_D, P, L = 64, 64, 100000, 4
N_CORES = 8
NT = 512                    # paths per inner tile
NTILES = 25                 # tiles per core
PC = NT * NTILES            # 12800 padded paths per core
PC_VALID = P // N_CORES     # 12500
HALF_A_TILES = 13           # tiles 0..12 use table A, 13..24 table B
TROWS = HALF_A_TILES * NT * L  # 26624 >= max distinct triples per half
BF16 = ml_dtypes.bfloat16

_CACHE = {}


def _build_kernel():
    nc = bacc.Bacc("TRN2", target_bir_lowering=False, debug=False, num_devices=N_CORES)
    dt = mybir.dt
    AF = mybir.ActivationFunctionType
    OP = mybir.AluOpType

    gtab = nc.dram_tensor("gtab", [2, TROWS, 256], dt.bfloat16, kind="ExternalInput")
    gidx = nc.dram_tensor("gidx", [128, NTILES * L * (NT // 16)], dt.int16, kind="ExternalInput")
    wxa = nc.dram_tensor("wxa", [128, 128], dt.bfloat16, kind="ExternalInput")
    wra = nc.dram_tensor("wra", [64, 128], dt.bfloat16, kind="ExternalInput")
    wha = nc.dram_tensor("wha", [64, 128], dt.bfloat16, kind="ExternalInput")
    wxb = nc.dram_tensor("wxb", [128, 128], dt.bfloat16, kind="ExternalInput")
    wrb = nc.dram_tensor("wrb", [64, 128], dt.bfloat16, kind="ExternalInput")
    whb = nc.dram_tensor("whb", [64, 128], dt.bfloat16, kind="ExternalInput")
    biasa = nc.dram_tensor("biasa", [128, 1], dt.float32, kind="ExternalInput")
    biasb = nc.dram_tensor("biasb", [128, 1], dt.float32, kind="ExternalInput")
    w1h = nc.dram_tensor("w1h", [64, 128], dt.bfloat16, kind="ExternalInput")
    w1c = nc.dram_tensor("w1c", [128, 128], dt.float32, kind="ExternalInput")
    attb1 = nc.dram_tensor("attb1", [128, 1], dt.float32, kind="ExternalInput")
    attw2 = nc.dram_tensor("attw2", [128, 1], dt.bfloat16, kind="ExternalInput")
    valw1 = nc.dram_tensor("valw1", [64, 32], dt.float32, kind="ExternalInput")
    valb1 = nc.dram_tensor("valb1", [32, 1], dt.float32, kind="ExternalInput")
    valw2 = nc.dram_tensor("valw2", [32, 1], dt.float32, kind="ExternalInput")
    safw1 = nc.dram_tensor("safw1", [128, 32], dt.float32, kind="ExternalInput")
    safb1 = nc.dram_tensor("safb1", [32, 1], dt.float32, kind="ExternalInput")
    safw2 = nc.dram_tensor("safw2", [32, 1], dt.float32, kind="ExternalInput")
    consts = nc.dram_tensor("consts", [1, 2], dt.float32, kind="ExternalInput")
    ctx = nc.dram_tensor("ctx", [128, 1], dt.float32, kind="ExternalInput")
    ident = nc.dram_tensor("ident", [64, 64], dt.bfloat16, kind="ExternalInput")
    ident128 = nc.dram_tensor("ident128", [128, 128], dt.float32, kind="ExternalInput")
    padoffs = nc.dram_tensor("padoffs", [128, NTILES * 4], dt.float32, kind="ExternalInput")

    w_out = nc.dram_tensor("w_out", [128, NTILES * 4], dt.float32, kind="ExternalOutput")
    pred_out = nc.dram_tensor("pred_out", [1, 1], dt.float32, kind="ExternalOutput")

    cin = nc.dram_tensor("cin", [1, 128], dt.float32, kind="Internal")
    cout = nc.dram_tensor("cout", [N_CORES, 128], dt.float32, kind="Internal", addr_space="Shared")

    with tile.TileContext(nc) as tc:
        from contextlib import ExitStack

        with ExitStack() as st:
            pp = st.enter_context(tc.tile_pool(name="persist", bufs=1))
            gp = st.enter_context(tc.tile_pool(name="gath", bufs=10))
            wp = st.enter_context(tc.tile_pool(name="work", bufs=3))
            sp = st.enter_context(tc.tile_pool(name="state", bufs=2))
            ap_ = st.enter_context(tc.tile_pool(name="att", bufs=2))
            pz = st.enter_context(tc.tile_pool(name="pz", bufs=2, space="PSUM"))
            pm = st.enter_context(tc.tile_pool(name="pm", bufs=1, space="PSUM"))
            pv = st.enter_context(tc.tile_pool(name="pv", bufs=1, space="PSUM"))

            def load(t, dram, dtype, shape):
                tl = pp.tile(shape, dtype, tag=t)
                nc.sync.dma_start(out=tl[:], in_=dram.ap())
                return tl

            wxa_t = load("wxa", wxa, dt.bfloat16, [128, 128])
            wra_t = load("wra", wra, dt.bfloat16, [64, 128])
            wha_t = load("wha", wha, dt.bfloat16, [64, 128])
            wxb_t = load("wxb", wxb, dt.bfloat16, [128, 128])
            wrb_t = load("wrb", wrb, dt.bfloat16, [64, 128])
            whb_t = load("whb", whb, dt.bfloat16, [64, 128])
            biasa_t = load("biasa", biasa, dt.float32, [128, 1])
            biasb_t = load("biasb", biasb, dt.float32, [128, 1])
            w1h_t = load("w1h", w1h, dt.bfloat16, [64, 128])
            w1c_t = load("w1c", w1c, dt.float32, [128, 128])
            attb1_t = load("attb1", attb1, dt.float32, [128, 1])
            attw2_t = load("attw2", attw2, dt.bfloat16, [128, 1])
            valw1_t = load("valw1", valw1, dt.float32, [64, 32])
            valb1_t = load("valb1", valb1, dt.float32, [32, 1])
            valw2_t = load("valw2", valw2, dt.float32, [32, 1])
            safw1_t = load("safw1", safw1, dt.float32, [128, 32])
            safb1_t = load("safb1", safb1, dt.float32, [32, 1])
            safw2_t = load("safw2", safw2, dt.float32, [32, 1])
            consts_t = load("consts", consts, dt.float32, [1, 2])
            ctx_t = load("ctx", ctx, dt.float32, [128, 1])
            ident_t = load("ident", ident, dt.bfloat16, [64, 64])
            ident128_t = load("ident128", ident128, dt.float32, [128, 128])
            padoffs_t = load("padoffs", padoffs, dt.float32, [128, NTILES * 4])
            gidx_t = load("gidx", gidx, dt.int16, [128, NTILES * L * (NT // 16)])

            h4all = pp.tile([64, PC], dt.bfloat16, tag="h4all")
            s_all = pp.tile([128, NTILES * 4], dt.float32, tag="s_all")
            nc.vector.memset(s_all[:], -1e30)

            # context part of attention hidden bias: cvec = W1c @ ctx + attb1
            cv_ps = pv.tile([128, 1], dt.float32, tag="pvs")
            nc.tensor.matmul(out=cv_ps[:], lhsT=w1c_t[:], rhs=ctx_t[:], start=True, stop=True)
            cvec = pp.tile([128, 1], dt.float32, tag="cvec")
            nc.vector.tensor_tensor(out=cvec[:], in0=cv_ps[:], in1=attb1_t[:], op=OP.add)

            # ---------------- LSTM over tiles ----------------
            for ti in range(NTILES):
                half = 0 if ti < HALF_A_TILES else 1
                c_prev = None
                h_prev = None
                for t in range(L):
                    g = gp.tile([128, 2, NT], dt.bfloat16, tag="g")
                    ib = (ti * L + t) * (NT // 16)
                    nc.gpsimd.dma_gather(
                        out_ap=g[:],
                        in_ap=gtab.ap()[half],
                        idxs_ap=gidx_t[:, ib : ib + NT // 16],
                        num_idxs=NT,
                        num_idxs_reg=NT,
                        elem_size=256,
                        transpose=True,
                    )
                    za = pz.tile([128, NT], dt.float32, tag="za")
                    zb = pz.tile([128, NT], dt.float32, tag="zb")
                    last = t > 0
                    nc.tensor.matmul(out=za[:], lhsT=wxa_t[:], rhs=g[:, 0, :], start=True, stop=False)
                    nc.tensor.matmul(out=za[:], lhsT=wra_t[:], rhs=g[0:64, 1, :], start=False, stop=not last)
                    if last:
                        nc.tensor.matmul(out=za[:], lhsT=wha_t[:], rhs=h_prev[:], start=False, stop=True)
                    nc.tensor.matmul(out=zb[:], lhsT=wxb_t[:], rhs=g[:, 0, :], start=True, stop=False)
                    nc.tensor.matmul(out=zb[:], lhsT=wrb_t[:], rhs=g[0:64, 1, :], start=False, stop=not last)
                    if last:
                        nc.tensor.matmul(out=zb[:], lhsT=whb_t[:], rhs=h_prev[:], start=False, stop=True)

                    IF = wp.tile([128, NT], dt.bfloat16, tag="IF")
                    nc.scalar.activation(out=IF[:], in_=za[:], func=AF.Sigmoid, bias=biasa_t[:])
                    GG = wp.tile([64, NT], dt.bfloat16, tag="GG")
                    nc.scalar.activation(out=GG[:], in_=zb[0:64, :], func=AF.Tanh, bias=biasb_t[0:64])
                    OO = wp.tile([64, NT], dt.bfloat16, tag="OO")
                    nc.scalar.activation(out=OO[:], in_=zb[64:128, :], func=AF.Sigmoid, bias=biasb_t[64:128])

                    c_new = sp.tile([128, NT], dt.bfloat16, tag="c")
                    if t == 0:
                        # c = i*g   (h0 = c0 = 0)
                        nc.vector.tensor_tensor(out=c_new[64:128, :], in0=IF[0:64, :], in1=GG[:], op=OP.mult)
                    else:
                        U = wp.tile([64, NT], dt.bfloat16, tag="U")
                        nc.vector.tensor_tensor(out=U[:], in0=IF[0:64, :], in1=GG[:], op=OP.mult)
                        V = wp.tile([64, NT], dt.bfloat16, tag="V")
                        nc.vector.tensor_tensor(out=V[:], in0=IF[64:128, :], in1=c_prev[64:128, :], op=OP.mult)
                        nc.vector.tensor_tensor(out=c_new[64:128, :], in0=U[:], in1=V[:], op=OP.add)
                    TC = wp.tile([64, NT], dt.bfloat16, tag="TC")
                    nc.scalar.activation(out=TC[:], in_=c_new[64:128, :], func=AF.Tanh)
                    if t == L - 1:
                        nc.vector.tensor_tensor(
                            out=h4all[:, ti * NT : (ti + 1) * NT], in0=OO[:], in1=TC[:], op=OP.mult
                        )
                    else:
                        h_new = sp.tile([64, NT], dt.bfloat16, tag="h")
                        nc.vector.tensor_tensor(out=h_new[:], in0=OO[:], in1=TC[:], op=OP.mult)
                        h_prev = h_new
                    c_prev = c_new

            # ---------------- attention scores (batch-major) ----------------
            for ti in range(NTILES):
                ah_ps = pm.tile([128, NT], dt.float32, tag="ahps")
                nc.tensor.matmul(
                    out=ah_ps[:], lhsT=w1h_t[:], rhs=h4all[:, ti * NT : (ti + 1) * NT], start=True, stop=True
                )
                AH = ap_.tile([128, NT], dt.bfloat16, tag="AH")
                nc.scalar.activation(out=AH[:], in_=ah_ps[:], func=AF.Relu, bias=cvec[:])
                sc = pv.tile([128, 4], dt.float32, tag="pvs")
                for c4 in range(4):
                    nc.tensor.matmul(
                        out=sc[:, c4 : c4 + 1],
                        lhsT=AH[:, c4 * 128 : (c4 + 1) * 128],
                        rhs=attw2_t[:],
                        start=True,
                        stop=True,
                    )
                nc.scalar.activation(out=s_all[:, ti * 4 : (ti + 1) * 4], in_=sc[:], func=AF.Copy)

            # mask padded paths: add host-provided offsets (-2e30 on pad entries)
            s_eff = pp.tile([128, NTILES * 4], dt.float32, tag="s_eff")
            nc.vector.tensor_tensor(out=s_eff[:], in0=s_all[:], in1=padoffs_t[:], op=OP.add)

            # ---------------- local softmax stats ----------------
            mx = pp.tile([128, 1], dt.float32, tag="mx")
            nc.vector.tensor_reduce(out=mx[:], in_=s_eff[:], axis=mybir.AxisListType.X, op=OP.max)
            trm_ps = pv.tile([1, 128], dt.float32, tag="pvs")
            nc.tensor.transpose(out=trm_ps[:], in_=mx[:], identity=ident128_t[:])
            m_loc = pp.tile([1, 1], dt.float32, tag="mloc")
            nc.vector.tensor_reduce(out=m_loc[:], in_=trm_ps[:], axis=mybir.AxisListType.X, op=OP.max)
            negm = pp.tile([128, 1], dt.float32, tag="negm")
            mb = pp.tile([128, 1], dt.float32, tag="mb")
            nc.gpsimd.partition_broadcast(mb[:], m_loc[:])
            nc.vector.tensor_scalar_mul(negm[:], mb[:], -1.0)

            E = pp.tile([128, NTILES * 4], dt.float32, tag="E")
            zp = pp.tile([128, 1], dt.float32, tag="zp")
            nc.scalar.activation(out=E[:], in_=s_eff[:], func=AF.Exp, bias=negm[:], accum_out=zp[:])
            Ebf = pp.tile([128, NTILES * 4], dt.bfloat16, tag="Ebf")
            nc.vector.tensor_copy(out=Ebf[:], in_=E[:])

            ones128 = pp.tile([128, 1], dt.float32, tag="ones128")
            nc.vector.memset(ones128[:], 1.0)
            z_ps = pv.tile([1, 1], dt.float32, tag="pvs")
            nc.tensor.matmul(out=z_ps[:], lhsT=zp[:], rhs=ones128[:], start=True, stop=True)
            z_loc = pp.tile([1, 1], dt.float32, tag="zloc")
            nc.scalar.activation(out=z_loc[:], in_=z_ps[:], func=AF.Copy)

            # V_loc[1,64] = sum_p w'_p h_p : transpose h to batch-major, then accumulate
            nchunks = PC // 128
            hbm_all = pp.tile([128, nchunks * 64], dt.bfloat16, tag="hbm_all")
            for c in range(nchunks):
                tr_ps = pm.tile([128, 64], dt.bfloat16, tag="trps")
                nc.tensor.transpose(
                    out=tr_ps[:], in_=h4all[:, c * 128 : (c + 1) * 128], identity=ident_t[:]
                )
                nc.vector.tensor_copy(out=hbm_all[:, c * 64 : (c + 1) * 64], in_=tr_ps[:])
            vt_ps = pv.tile([1, 64], dt.float32, tag="pvs")
            for c in range(nchunks):
                nc.tensor.matmul(
                    out=vt_ps[:],
                    lhsT=Ebf[:, c : c + 1],
                    rhs=hbm_all[:, c * 64 : (c + 1) * 64],
                    start=(c == 0),
                    stop=(c == nchunks - 1),
                )

            # ---------------- allgather of (V, m, Z) ----------------
            sin = pp.tile([1, 128], dt.float32, tag="sin")
            nc.vector.memset(sin[:], 0.0)
            nc.scalar.activation(out=sin[:, 0:64], in_=vt_ps[:], func=AF.Copy)
            nc.scalar.activation(out=sin[:, 64:65], in_=m_loc[:], func=AF.Copy)
            nc.scalar.activation(out=sin[:, 65:66], in_=z_loc[:], func=AF.Copy)
            nc.sync.dma_start(out=cin.ap(), in_=sin[:])
            nc.gpsimd.collective_compute(
                "AllGather",
                OP.bypass,
                replica_groups=[list(range(N_CORES))],
                ins=[cin.ap()],
                outs=[cout.ap()],
            )
            ST = pp.tile([N_CORES, 128], dt.float32, tag="ST")
            nc.sync.dma_start(out=ST[:], in_=cout.ap())

            # ---------------- global combine ----------------
            tr8_ps = pv.tile([1, 8], dt.float32, tag="pvs")
            nc.tensor.transpose(out=tr8_ps[:], in_=ST[:, 64:65], identity=ident128_t[0:8, 0:8])
            mg = pp.tile([1, 1], dt.float32, tag="mg")
            nc.vector.tensor_reduce(out=mg[:], in_=tr8_ps[:], axis=mybir.AxisListType.X, op=OP.max)
            nmg = pp.tile([1, 1], dt.float32, tag="nmg")
            nc.vector.tensor_scalar_mul(nmg[:], mg[:], -1.0)
            nmg8 = pp.tile([N_CORES, 1], dt.float32, tag="nmg8")
            nc.gpsimd.partition_broadcast(nmg8[:], nmg[:])
            e8 = pp.tile([N_CORES, 1], dt.float32, tag="e8")
            nc.scalar.activation(out=e8[:], in_=ST[:, 64:65], func=AF.Exp, bias=nmg8[:])

            ones8 = pp.tile([N_CORES, 1], dt.float32, tag="ones8")
            nc.vector.memset(ones8[:], 1.0)
            zw = pp.tile([N_CORES, 1], dt.float32, tag="zw")
            nc.vector.tensor_tensor(out=zw[:], in0=ST[:, 65:66], in1=e8[:], op=OP.mult)
            zg_ps = pv.tile([1, 1], dt.float32, tag="pvs")
            nc.tensor.matmul(out=zg_ps[:], lhsT=zw[:], rhs=ones8[:], start=True, stop=True)
            zg_sb = pp.tile([1, 1], dt.float32, tag="zgsb")
            nc.scalar.activation(out=zg_sb[:], in_=zg_ps[:], func=AF.Copy)

            ve = pp.tile([N_CORES, 64], dt.float32, tag="ve")
            nc.vector.tensor_scalar_mul(ve[:], ST[:, 0:64], e8[:])
            vg_ps = pv.tile([64, 1], dt.float32, tag="pvs")
            nc.tensor.matmul(out=vg_ps[:], lhsT=ve[:], rhs=ones8[:], start=True, stop=True)
            vg = pp.tile([64, 1], dt.float32, tag="vg")
            nc.scalar.activation(out=vg[:], in_=vg_ps[:], func=AF.Copy)

            # w output scale = exp(m_loc - m_g) / Z_g
            eo = pp.tile([1, 1], dt.float32, tag="eo")
            nc.scalar.activation(out=eo[:], in_=m_loc[:], func=AF.Exp, bias=nmg[:])
            rz = pp.tile([1, 1], dt.float32, tag="rz")
            nc.vector.reciprocal(out=rz[:], in_=zg_sb[:])
            ws = pp.tile([1, 1], dt.float32, tag="ws")
            nc.vector.tensor_tensor(out=ws[:], in0=eo[:], in1=rz[:], op=OP.mult)
            ws128 = pp.tile([128, 1], dt.float32, tag="ws128")
            nc.gpsimd.partition_broadcast(ws128[:], ws[:])
            rz64 = pp.tile([64, 1], dt.float32, tag="rz64")
            nc.gpsimd.partition_broadcast(rz64[:], rz[:])
            vgn = pp.tile([64, 1], dt.float32, tag="vgn")
            nc.vector.tensor_scalar_mul(vgn[:], vg[:], rz64[:])
            wout_t = pp.tile([128, NTILES * 4], dt.float32, tag="wout")
            nc.vector.tensor_scalar_mul(wout_t[:], E[:], ws128[:])
            nc.sync.dma_start(out=w_out.ap(), in_=wout_t[:])

            # value head on aggregated = vg
            hv_ps = pv.tile([32, 1], dt.float32, tag="pvs")
            nc.tensor.matmul(out=hv_ps[:], lhsT=valw1_t[:], rhs=vgn[:], start=True, stop=True)
            hv = pp.tile([32, 1], dt.float32, tag="hv")
            nc.scalar.activation(out=hv[:], in_=hv_ps[:], func=AF.Relu, bias=valb1_t[:])
            val_ps = pv.tile([1, 1], dt.float32, tag="pvs")
            nc.tensor.matmul(out=val_ps[:], lhsT=valw2_t[:], rhs=hv[:], start=True, stop=True)
            q = pp.tile([1, 1], dt.float32, tag="q")
            nc.vector.tensor_tensor(out=q[:], in0=val_ps[:], in1=consts_t[:, 0:1], op=OP.add)

            # safety head on ctx
            hs_ps = pv.tile([32, 1], dt.float32, tag="pvs")
            nc.tensor.matmul(out=hs_ps[:], lhsT=safw1_t[:], rhs=ctx_t[:], start=True, stop=True)
            hs = pp.tile([32, 1], dt.float32, tag="hs")
            nc.scalar.activation(out=hs[:], in_=hs_ps[:], func=AF.Relu, bias=safb1_t[:])
            s2_ps = pv.tile([1, 1], dt.float32, tag="pvs")
            nc.tensor.matmul(out=s2_ps[:], lhsT=safw2_t[:], rhs=hs[:], start=True, stop=True)
            sf = pp.tile([1, 1], dt.float32, tag="sf")
            nc.scalar.activation(out=sf[:], in_=s2_ps[:], func=AF.Sigmoid, bias=consts_t[:, 1:2])

            pred_t = pp.tile([1, 1], dt.float32, tag="pred")
            nc.vector.tensor_tensor(out=pred_t[:], in0=q[:], in1=sf[:], op=OP.mult)
            nc.sync.dma_start(out=pred_out.ap(), in_=pred_t[:])

    nc.compile()
    return nc


def _padoffs():
    o = np.zeros((128, NTILES * 4), np.float32)
    o[84:, 97] = -2e30
    o[:, 98:] = -2e30
    return o


def _prep_host(inputs):
    """Build per-core input maps from full inputs."""
    f32 = np.float32
    user_emb = np.asarray(inputs["user_emb"], f32)
    item_emb = np.asarray(inputs["item_emb"], f32)
    relation_emb = np.asarray(inputs["relation_emb"], f32)
    node_type_emb = np.asarray(inputs["node_type_emb"], f32)
    node_ids = np.asarray(inputs["node_ids"]).astype(np.int64)
    node_types = np.asarray(inputs["node_types"]).astype(np.int64)
    rel_idx = np.asarray(inputs["rel_idx"]).astype(np.int64)
    W_ih = np.asarray(inputs["W_ih"], f32)
    W_hh = np.asarray(inputs["W_hh"], f32)
    b = np.asarray(inputs["b_ih"], f32) + np.asarray(inputs["b_hh"], f32)
    att_W1 = np.asarray(inputs["att_W1"], f32)
    att_b1 = np.asarray(inputs["att_b1"], f32)
    att_W2 = np.asarray(inputs["att_W2"], f32)
    val_W1 = np.asarray(inputs["val_W1"], f32)
    val_b1 = np.asarray(inputs["val_b1"], f32)
    val_W2 = np.asarray(inputs["val_W2"], f32)
    val_b2 = np.asarray(inputs["val_b2"], f32)
    saf_W1 = np.asarray(inputs["saf_W1"], f32)
    saf_b1 = np.asarray(inputs["saf_b1"], f32)
    saf_W2 = np.asarray(inputs["saf_W2"], f32)
    saf_b2 = np.asarray(inputs["saf_b2"], f32)
    uidx = int(np.asarray(inputs["user_idx"]).reshape(-1)[0])
    iidx = int(np.asarray(inputs["item_idx"]).reshape(-1)[0])

    shared = {
        "wxa": np.ascontiguousarray(W_ih[0:128, 0:128].T).astype(BF16),
        "wra": np.ascontiguousarray(W_ih[0:128, 128:192].T).astype(BF16),
        "wha": np.ascontiguousarray(W_hh[0:128, :].T).astype(BF16),
        "wxb": np.ascontiguousarray(W_ih[128:256, 0:128].T).astype(BF16),
        "wrb": np.ascontiguousarray(W_ih[128:256, 128:192].T).astype(BF16),
        "whb": np.ascontiguousarray(W_hh[128:256, :].T).astype(BF16),
        "biasa": b[0:128].reshape(128, 1).copy(),
        "biasb": b[128:256].reshape(128, 1).copy(),
        "w1h": np.ascontiguousarray(att_W1[:, 0:64].T).astype(BF16),
        "w1c": np.ascontiguousarray(att_W1[:, 64:192].T).astype(f32),
        "attb1": att_b1.reshape(128, 1).copy(),
        "attw2": np.ascontiguousarray(att_W2.T).astype(BF16),
        "valw1": np.ascontiguousarray(val_W1.T).astype(f32),
        "valb1": val_b1.reshape(32, 1).copy(),
        "valw2": np.ascontiguousarray(val_W2.T).astype(f32),
        "safw1": np.ascontiguousarray(saf_W1.T).astype(f32),
        "safb1": saf_b1.reshape(32, 1).copy(),
        "safw2": np.ascontiguousarray(saf_W2.T).astype(f32),
        "consts": np.array([[val_b2.reshape(-1)[0], saf_b2.reshape(-1)[0]]], f32),
        "ctx": np.concatenate([user_emb[uidx], item_emb[iidx]]).reshape(128, 1).astype(f32),
        "ident": np.eye(64, dtype=np.float32).astype(BF16),
        "ident128": np.eye(128, dtype=np.float32),
        "padoffs": _padoffs(),
    }

    in_maps = []
    for core in range(N_CORES):
        lo = core * PC_VALID
        nid = node_ids[lo : lo + PC_VALID]
        nty = node_types[lo : lo + PC_VALID]
        rel = rel_idx[lo : lo + PC_VALID]
        pad = PC - PC_VALID
        nid = np.concatenate([nid, np.repeat(nid[:1], pad, 0)], 0)
        nty = np.concatenate([nty, np.repeat(nty[:1], pad, 0)], 0)
        rel = np.concatenate([rel, np.repeat(rel[:1], pad, 0)], 0)
        key = (nid + 50000 * nty + 100000 * rel).astype(np.int64)  # [PC, L]

        gtab_np = np.zeros((2, TROWS, 256), BF16)
        idx16 = np.zeros((PC, L), np.int16)
        bounds = [(0, HALF_A_TILES * NT), (HALF_A_TILES * NT, PC)]
        for h, (a0, a1) in enumerate(bounds):
            k = key[a0:a1]
            uk, inv = np.unique(k, return_inverse=True)
            n = len(uk)
            t_ = (uk // 50000) % 2
            i_ = uk % 50000
            r_ = uk // 100000
            node_e = np.where(t_[:, None] == 0, user_emb[i_], item_emb[i_])
            gtab_np[h, :n, 0:64] = node_e.astype(BF16)
            gtab_np[h, :n, 64:128] = node_type_emb[t_].astype(BF16)
            gtab_np[h, :n, 128:192] = relation_emb[r_].astype(BF16)
            idx16[a0:a1] = inv.reshape(k.shape).astype(np.int16)

        # pack indices: per (tile, step) a [128, 32] int16 block
        gidx_np = np.zeros((128, NTILES * L * (NT // 16)), np.int16)
        for ti in range(NTILES):
            for t in range(L):
                chunk = idx16[ti * NT : (ti + 1) * NT, t]
                blk = np.tile(chunk.reshape(NT // 16, 16).T, (8, 1))
                cb = (ti * L + t) * (NT // 16)
                gidx_np[:, cb : cb + NT // 16] = blk

        m = dict(shared)
        m["gtab"] = gtab_np
        m["gidx"] = gidx_np
        in_maps.append(m)
    return in_maps


def kernel(**inputs):
    if "nc" not in _CACHE:
        _CACHE["nc"] = _build_kernel()
    nc = _CACHE["nc"]
    in_maps = _prep_host(inputs)
    res = bass_utils.run_bass_kernel_spmd(nc, in_maps, core_ids=list(range(N_CORES)))
    _CACHE["last_res"] = res

    att = np.zeros((P, 1), np.float32)
    p = np.arange(PC_VALID)
    rows = p % 128
    cols = (p // NT) * 4 + (p % NT) // 128
    for core in range(N_CORES):
        W = res.results[core]["w_out"]
        att[core * PC_VALID : (core + 1) * PC_VALID, 0] = W[rows, cols]
    pred = res.results[0]["pred_out"].astype(np.float32)
    return pred, att
